# revision 42
# baseline (speedup 1.0000x reference)
"""GQA attention kernel for 8 Trainium2 NeuronCores.

Head-parallel sharding: core c owns q-heads [4c, 4c+4) and kv-head c.
Each core computes its 4 heads' attention and a partial output
projection (row-parallel wo); the host sums the 8 partials (bf16).

v2: globally software-pipelined schedule. The PE-side work of batch
b1's QKV projection and both batches' output projections is chopped
into small "filler" items that are interleaved into the exp-gated
attention chunk streams, so the PE never idles (keeping its DVFS
clock at max). Diagonal score chunks restrict their moving range to
the causally-needed columns (saves ~15% of attention PE+exp work and
removes all pt zero-memsets). The softmax reciprocal uses the fast
approx DVE op instead of the 3.4us exact reciprocal, and all
evictions/masking run on Pool/Vector so Scalar does exps only.
"""

import sys

sys.path.insert(0, "/opt/trn_rl_repo")

import numpy as np
import ml_dtypes

import concourse.bacc as bacc
import concourse.bass as bass
import concourse.mybir as mybir
from concourse import tile
from concourse.bass_utils import run_bass_kernel_spmd
from concourse.masks import make_identity

B, T, D = 2, 2048, 2048
H, HKV, HD = 32, 8, 64
NCORE = 8
HLOC = H // NCORE          # 4 q heads per core
QCOLS = HLOC * HD          # 256
NB = T // 512              # token nblocks per batch
KC = D // 128              # contraction chunks for projections
LEAD = 2                   # AV chunks lag scores by this many chunks

F32 = mybir.dt.float32
BF16 = mybir.dt.bfloat16

SWAP_MASK = [i ^ 1 for i in range(32)]  # adjacent pair swap per quadrant

EXP_FN = mybir.ActivationFunctionType.Exp


def build_nc():
    nc = bacc.Bacc(None, target_bir_lowering=False, debug=False)

    xT = nc.dram_tensor("xT", [D, B * T], BF16, kind="ExternalInput")
    wq_d = nc.dram_tensor("wq", [D, QCOLS], BF16, kind="ExternalInput")
    wkv_d = nc.dram_tensor("wkv", [D, 2 * HD], BF16, kind="ExternalInput")
    wo_d = nc.dram_tensor("wo", [QCOLS, D], BF16, kind="ExternalInput")
    rAq_d = nc.dram_tensor("ropeAq", [128, T], BF16, kind="ExternalInput")
    rBq_d = nc.dram_tensor("ropeBq", [128, T], BF16, kind="ExternalInput")
    rAkv_d = nc.dram_tensor("ropeAkv", [128, T], BF16, kind="ExternalInput")
    rBkv_d = nc.dram_tensor("ropeBkv", [128, T], BF16, kind="ExternalInput")
    out_d = nc.dram_tensor("out", [B * T, D], BF16, kind="ExternalOutput")

    with tile.TileContext(nc) as tc:
        with (
            tc.tile_pool(name="consts", bufs=1) as consts,
            tc.tile_pool(name="xp", bufs=36) as xp,
            tc.tile_pool(name="shufp", bufs=3) as shufp,
            tc.tile_pool(name="ropea", bufs=3) as ropea,
            tc.tile_pool(name="ropeb", bufs=3) as ropeb,
            tc.tile_pool(name="qropep", bufs=8) as qropep,
            tc.tile_pool(name="kvropep", bufs=2) as kvropep,
            tc.tile_pool(name="vextp", bufs=2) as vextp,
            tc.tile_pool(name="ptp", bufs=10) as ptp,
            tc.tile_pool(name="recipp", bufs=4) as recipp,
            tc.tile_pool(name="rbcp", bufs=3) as rbcp,
            tc.tile_pool(name="attp", bufs=4) as attp,
            tc.tile_pool(name="outp", bufs=6) as outp,
            tc.tile_pool(name="spp", bufs=3, space="PSUM") as spp,
            tc.tile_pool(name="avp", bufs=2, space="PSUM") as avp,
            tc.tile_pool(name="fillp", bufs=3, space="PSUM") as fillp,
        ):
            # ---- constants ----
            wq_sb = consts.tile([128, KC, QCOLS], BF16)
            nc.sync.dma_start(
                out=wq_sb[:], in_=wq_d.rearrange("(kc p) m -> p kc m", p=128)
            )
            wkv_sb = consts.tile([128, KC, 2 * HD], BF16)
            nc.sync.dma_start(
                out=wkv_sb[:], in_=wkv_d.rearrange("(kc p) m -> p kc m", p=128)
            )
            # wo/rope tiles are allocated here but their DMAs are issued
            # after the first x-tile batch (they aren't needed immediately
            # and would delay the first projection matmuls)
            wo_sb = consts.tile([128, 2, D], BF16)
            rAq = consts.tile([128, T], BF16)
            rBq = consts.tile([128, T], BF16)
            rAkv = consts.tile([128, T], BF16)
            rBkv = consts.tile([128, T], BF16)

            def emit_late_const_dmas():
                nc.sync.dma_start(out=rAq[:], in_=rAq_d[:])
                nc.sync.dma_start(out=rBq[:], in_=rBq_d[:])
                nc.sync.dma_start(out=rAkv[:], in_=rAkv_d[:])
                nc.sync.dma_start(out=rBkv[:], in_=rBkv_d[:])
                nc.sync.dma_start(
                    out=wo_sb[:], in_=wo_d.rearrange("(g p) n -> p g n", p=128)
                )

            ident = consts.tile([128, 128], BF16)
            make_identity(nc, ident[:])
            # mnegT[q, k] = -30000 if k > q else 0 — stationary operand of the
            # causal-mask matmul (adds -30000 above the diagonal of scores)
            mnegT = consts.tile([128, 128], BF16)
            nc.gpsimd.memset(mnegT[:], -30000.0)
            nc.gpsimd.affine_select(
                out=mnegT[:],
                in_=mnegT[:],
                compare_op=mybir.AluOpType.is_ge,
                fill=0.0,
                base=-1,
                pattern=[[1, 128]],
                channel_multiplier=-1,
            )
            # ident_ext: [I | 0] — moving operand of the causal-mask matmul
            ident_ext = consts.tile([128, 512], BF16)
            nc.gpsimd.memset(ident_ext[:], 0.0)
            nc.gpsimd.tensor_copy(ident_ext[:, 0:128], ident[:])


            # ---- per-batch persistent tiles ----
            qrope = {}   # (b, h) -> [64, T]
            kvrope = {}  # b -> [128, T]
            v_ext = {}   # b -> [128, KC, HD+1]
            attT = {}    # (b, g) -> [128, T]
            xts = {}     # (b, nb) -> list of 16 [128,512] tiles

            def alloc_batch(b):
                for h in range(HLOC):
                    qrope[(b, h)] = qropep.tile([64, T], BF16, tag="qrope", name=f"qr{b}{h}")
                kvrope[b] = kvropep.tile([128, T], BF16, tag="kvrope", name=f"kv{b}")
                for g in range(2):
                    attT[(b, g)] = attp.tile([128, T], BF16, tag="attT", name=f"at{b}{g}")

            alloc_batch(0)
            alloc_batch(1)

            # ---------- filler item emitters ----------
            def emit_dma_x(b, nb):
                tiles = []
                c0 = b * T + nb * 512
                for kc in range(KC):
                    xt = xp.tile([128, 512], BF16, tag="x", name="x")
                    nc.sync.dma_start(
                        out=xt[:], in_=xT[kc * 128 : (kc + 1) * 128, c0 : c0 + 512]
                    )
                    tiles.append(xt)
                xts[(b, nb)] = tiles

            def emit_proj_half(b, nb, mt, half, ps):
                # mt: 0,1 -> q cols [128*mt,128*mt+128); 2 -> kv
                tiles = xts[(b, nb)]
                for kc in range(half * 8, half * 8 + 8):
                    if mt < 2:
                        lhsT = wq_sb[:, kc, mt * 128 : (mt + 1) * 128]
                    else:
                        lhsT = wkv_sb[:, kc, :]
                    nc.tensor.matmul(
                        ps[:],
                        lhsT,
                        tiles[kc][:],
                        start=(kc == 0),
                        stop=(kc == KC - 1),
                    )

            def emit_rope_evict(b, nb, mt, ps):
                A = rAq if mt < 2 else rAkv
                Bp = rBq if mt < 2 else rBkv
                sl = slice(nb * 512, (nb + 1) * 512)
                tmp = shufp.tile([128, 512], F32, tag="shuf", name="shuf")
                nc.vector.stream_shuffle(tmp[:], ps[:], SWAP_MASK)
                t2 = ropea.tile([128, 512], BF16, tag="ra", name="ra")
                nc.vector.tensor_mul(t2[:], ps[:], A[:, sl])
                t3 = ropeb.tile([128, 512], BF16, tag="rb", name="rb")
                nc.gpsimd.tensor_mul(t3[:], tmp[:], Bp[:, sl])
                if mt < 2:
                    nc.gpsimd.tensor_add(
                        qrope[(b, 2 * mt)][:, sl], t2[0:64, :], t3[0:64, :]
                    )
                    nc.gpsimd.tensor_add(
                        qrope[(b, 2 * mt + 1)][:, sl], t2[64:128, :], t3[64:128, :]
                    )
                else:
                    nc.gpsimd.tensor_add(kvrope[b][:, sl], t2[:], t3[:])

            def emit_vext_nb(b, nb):
                # transpose V rows of this nb's 4 key chunks into one PSUM
                # bank, then evict with a single strided copy
                ve = v_ext[b]
                tp = fillp.tile([128, 4, HD], BF16, tag="fill", name="tpv")
                for jj in range(4):
                    j = 4 * nb + jj
                    nc.tensor.transpose(
                        tp[:, jj, :],
                        kvrope[b][HD:128, j * 128 : (j + 1) * 128],
                        ident[HD:128, HD:128],
                    )
                nc.scalar.activation(
                    ve[:, 4 * nb : 4 * nb + 4, 0:HD],
                    tp[:],
                    mybir.ActivationFunctionType.Copy,
                )

            _evict_flip = [0]

            def emit_outproj_tile(b, mt, nb2):
                op = fillp.tile([128, 512], F32, tag="fill", name="op")
                for g in range(2):
                    nc.tensor.matmul(
                        op[:],
                        attT[(b, g)][:, mt * 128 : (mt + 1) * 128],
                        wo_sb[:, g, nb2 * 512 : (nb2 + 1) * 512],
                        start=(g == 0),
                        stop=(g == 1),
                    )
                ot = outp.tile([128, 512], BF16, tag="ot", name="ot")
                if _evict_flip[0] % 4 == 3:
                    nc.scalar.activation(
                        ot[:], op[:], mybir.ActivationFunctionType.Copy
                    )
                else:
                    nc.vector.tensor_copy(ot[:], op[:])
                _evict_flip[0] += 1
                r0 = b * T + mt * 128
                nc.sync.dma_start(
                    out=out_d[r0 : r0 + 128, nb2 * 512 : (nb2 + 1) * 512],
                    in_=ot[:],
                )

            # ---------- filler queues ----------
            # each item: (pe_ns_estimate, fn)
            fill_proj = []   # must drain before attention of that batch
            fill_op = []     # output-projection tiles, can linger

            def build_proj_items(b):
                # items: (pe_ns, fn, stage) — attention(b, qb) may start once
                # all items with stage <= b*NB + qb have been emitted
                items = []
                chains = {}

                def mk_dma(nb, stage):
                    return (100.0, (lambda nb=nb: emit_dma_x(b, nb)), stage)

                def mk_half(nb, mt, half):
                    def f(nb=nb, mt=mt, half=half):
                        if (nb, mt) not in chains:
                            chains[(nb, mt)] = fillp.tile(
                                [128, 512], F32, tag="fill", name="pj"
                            )
                        emit_proj_half(b, nb, mt, half, chains[(nb, mt)])
                    return (8 * 512 * 0.42, f, b * NB + nb)

                def mk_evict(nb, mt):
                    def f(nb=nb, mt=mt):
                        emit_rope_evict(b, nb, mt, chains.pop((nb, mt)))
                    return (50.0, f, b * NB + nb)

                def mk_vext(nb):
                    return (300.0, (lambda nb=nb: emit_vext_nb(b, nb)), b * NB + nb)

                items.append(mk_dma(0, b * NB))
                if b == 0:
                    items.append((100.0, emit_late_const_dmas, 0))
                items.append(mk_dma(1, b * NB))
                for nb in range(NB):
                    if nb >= 1 and nb + 1 < NB:
                        items.append(mk_dma(nb + 1, b * NB + nb))
                    # kv chain first so its rope adds drain while q0/q1 run,
                    # letting the v_ext transposes start stall-free
                    for mt in (2, 0, 1):
                        items.append(mk_half(nb, mt, 0))
                        items.append(mk_half(nb, mt, 1))
                        items.append(mk_evict(nb, mt))
                    items.append(mk_vext(nb))
                return items

            deficit = [0.0]

            def pull():
                # emit filler items worth ~deficit ns of PE time
                budget = deficit[0]
                while (fill_proj or fill_op) and budget > 0:
                    if fill_proj:
                        pe_ns, fn, _ = fill_proj[0]
                        q = fill_proj
                    else:
                        pe_ns, fn = fill_op[0]
                        q = fill_op
                    if pe_ns > budget + 400:
                        break
                    q.pop(0)
                    fn()
                    budget -= pe_ns
                    deficit[0] -= pe_ns

            def drain_proj_stage(stage):
                while fill_proj and fill_proj[0][2] <= stage:
                    _, fn, _ = fill_proj.pop(0)
                    fn()
                    if fill_op:
                        pe_ns, fn2 = fill_op.pop(0)
                        fn2()
                        deficit[0] -= pe_ns

            # ---------- attention ----------
            def emit_attention(b):
                ve = v_ext[b]
                kvr = kvrope[b]
                pending_norm = []
                norms_done = [0]

                def flush_one_norm():
                    if not pending_norm:
                        return
                    pending_norm.pop(0)()
                    norms_done[0] += 1
                    if norms_done[0] % HLOC == 0:
                        qb_c = norms_done[0] // HLOC - 1
                        for mt in range(4 * qb_c, 4 * qb_c + 4):
                            for nb2 in range(NB):
                                fill_op.append(
                                    (2 * 512 * 0.42 + 80,
                                     lambda b=b, mt=mt, nb2=nb2:
                                         emit_outproj_tile(b, mt, nb2))
                                )

                for qb in range(NB):
                    drain_proj_stage(b * NB + qb)
                    q0 = qb * 512
                    for h in range(HLOC):
                        qTh = qrope[(b, h)]
                        nch = 4 * qb + 4
                        av = avp.tile([128, 512], F32, tag="av", name="av")
                        pend = []

                        def emit_av(j, mc, pt):
                            nc.tensor.matmul(
                                av[0 : HD + 1, mc:512],
                                ve[:, j, :],
                                pt[:, mc:512],
                                start=(j == 0),
                                stop=(j == nch - 1),
                            )

                        for j in range(nch):
                            jj = j - 4 * qb
                            mc = 128 * jj if jj > 0 else 0
                            sp = spp.tile([128, 512], F32, tag="sp", name="sp")
                            if jj >= 0:
                                # causal mask: sp[k, mc:512] = -30000*(k > q)
                                nc.tensor.matmul(
                                    sp[:, mc:512],
                                    mnegT[:],
                                    ident_ext[:, 0 : 512 - mc],
                                    start=True,
                                    stop=False,
                                )
                            nc.tensor.matmul(
                                sp[:, mc:512],
                                kvr[0:HD, j * 128 : (j + 1) * 128],
                                qTh[:, q0 + mc : q0 + 512],
                                start=(jj < 0),
                                stop=True,
                            )
                            pt = ptp.tile([128, 512], BF16, tag="pt", name="pt")
                            nc.scalar.activation(
                                pt[:, mc:512], sp[:, mc:512], EXP_FN
                            )
                            pend.append((j, mc, pt))
                            cols = 512 - mc
                            deficit[0] += (0.83 * cols + 280) - (0.84 * cols + 60) + 120
                            if j == 1:
                                flush_one_norm()
                            if len(pend) > LEAD:
                                emit_av(*pend.pop(0))
                            pull()
                        while pend:
                            emit_av(*pend.pop(0))
                        # normalize via fast reciprocal of the sums row
                        # (staged to SBUF: the approx reciprocal's bit tricks
                        # need SBUF fp32), then a rank-1 PE matmul broadcasts
                        # the reciprocal into the av bank's free rows 64:128.
                        # The reciprocal runs now (vector); the PE-side
                        # broadcast + final mul are deferred into the next
                        # head's chunk stream so the PE never waits on them.
                        rcs = recipp.tile([1, 512], F32, tag="recips", name="rcs")
                        nc.vector.tensor_copy(rcs[:], av[HD : HD + 1, :])
                        rct = recipp.tile([1, 512], F32, tag="recip", name="rc")
                        nc.vector.reciprocal_approx_fast(rct[:], rcs[:])
                        rb = rbcp.tile([HD, 512], F32, tag="rbc", name="rb")
                        nc.gpsimd.partition_broadcast(rb[:], rct[:], channels=HD)

                        def norm_fn(av=av, rb=rb, b=b, h=h, q0=q0):
                            dest = attT[(b, h // 2)][
                                HD * (h % 2) : HD * (h % 2) + HD, q0 : q0 + 512
                            ]
                            nc.vector.tensor_mul(dest, av[0:HD, :], rb[:])

                        pending_norm.append(norm_fn)
                while pending_norm:
                    flush_one_norm()

            # ---------- the global schedule ----------
            # One merged stream: attention(b, qb) is emitted as soon as the
            # projections for its token blocks are in; everything else
            # (later proj chains, output projections) fills PE idle slots.
            v_ext[0] = vextp.tile([128, KC, HD + 1], BF16, tag="vext", name="ve0")
            nc.gpsimd.memset(v_ext[0][:], 1.0)
            v_ext[1] = vextp.tile([128, KC, HD + 1], BF16, tag="vext", name="ve1")
            nc.gpsimd.memset(v_ext[1][:], 1.0)

            fill_proj.extend(build_proj_items(0))
            fill_proj.extend(build_proj_items(1))
            emit_attention(0)
            emit_attention(1)

            # drain the rest
            while fill_op:
                _, fn = fill_op.pop(0)
                fn()

    nc.compile()
    return nc


_NC = None


def _get_nc():
    global _NC
    if _NC is None:
        _NC = build_nc()
    return _NC


def make_in_maps(x, freqs_cos, freqs_sin, wq, wk, wv, wo):
    npdt = ml_dtypes.bfloat16
    x = np.asarray(x, np.float32)
    freqs_cos = np.asarray(freqs_cos, np.float32)
    freqs_sin = np.asarray(freqs_sin, np.float32)
    wq = np.asarray(wq, np.float32)
    wk = np.asarray(wk, np.float32)
    wv = np.asarray(wv, np.float32)
    wo = np.asarray(wo, np.float32)

    xT = np.ascontiguousarray(x.reshape(B * T, D).T.astype(npdt))

    cosT = freqs_cos.T  # [32, T]
    sinT = freqs_sin.T
    A64 = np.empty((64, T), np.float32)
    A64[0::2] = cosT
    A64[1::2] = cosT
    B64 = np.empty((64, T), np.float32)
    B64[0::2] = -sinT
    B64[1::2] = sinT
    one64 = np.ones((64, T), np.float32)
    zero64 = np.zeros((64, T), np.float32)
    rAq = np.ascontiguousarray(np.concatenate([A64, A64], 0).astype(npdt))
    rBq = np.ascontiguousarray(np.concatenate([B64, B64], 0).astype(npdt))
    rAkv = np.ascontiguousarray(np.concatenate([A64, one64], 0).astype(npdt))
    rBkv = np.ascontiguousarray(np.concatenate([B64, zero64], 0).astype(npdt))

    scale = np.float32(1.0 / np.sqrt(HD))
    in_maps = []
    for c in range(NCORE):
        wq_c = np.ascontiguousarray((wq[:, c * QCOLS : (c + 1) * QCOLS] * scale).astype(npdt))
        wkv_c = np.ascontiguousarray(
            np.concatenate(
                [wk[:, c * HD : (c + 1) * HD], wv[:, c * HD : (c + 1) * HD]], 1
            ).astype(npdt)
        )
        wo_c = np.ascontiguousarray(wo[c * QCOLS : (c + 1) * QCOLS, :].astype(npdt))
        in_maps.append(
            {
                "xT": xT,
                "wq": wq_c,
                "wkv": wkv_c,
                "wo": wo_c,
                "ropeAq": rAq,
                "ropeBq": rBq,
                "ropeAkv": rAkv,
                "ropeBkv": rBkv,
            }
        )
    return in_maps


def run(in_maps, trace=False, **kwargs):
    nc = _get_nc()
    return run_bass_kernel_spmd(
        nc, in_maps, core_ids=list(range(NCORE)), trace=trace, **kwargs
    )


def kernel(x, freqs_cos, freqs_sin, wq, wk, wv, wo):
    in_maps = make_in_maps(x, freqs_cos, freqs_sin, wq, wk, wv, wo)
    res = run(in_maps)
    total = np.zeros((B * T, D), np.float32)
    for r in res.results:
        total += np.asarray(r["out"], np.float32)
    return total.reshape(B, T, D)


# revision 44
# speedup vs baseline: 1.1731x; 1.1731x over previous
"""GQA attention kernel for 8 Trainium2 NeuronCores.

Head-parallel sharding: core c owns q-heads [4c, 4c+4) and kv-head c.
Each core computes its 4 heads' attention and a partial output
projection (row-parallel wo); the host sums the 8 partials (bf16).

v2: globally software-pipelined schedule. The PE-side work of batch
b1's QKV projection and both batches' output projections is chopped
into small "filler" items that are interleaved into the exp-gated
attention chunk streams, so the PE never idles (keeping its DVFS
clock at max). Diagonal score chunks restrict their moving range to
the causally-needed columns (saves ~15% of attention PE+exp work and
removes all pt zero-memsets). The softmax reciprocal uses the fast
approx DVE op instead of the 3.4us exact reciprocal, and all
evictions/masking run on Pool/Vector so Scalar does exps only.
"""

import sys

sys.path.insert(0, "/opt/trn_rl_repo")

import numpy as np
import ml_dtypes

import concourse.bacc as bacc
import concourse.bass as bass
import concourse.mybir as mybir
from concourse import tile
from concourse.bass_utils import run_bass_kernel_spmd
from concourse.masks import make_identity

B, T, D = 2, 2048, 2048
H, HKV, HD = 32, 8, 64
NCORE = 8
HLOC = H // NCORE          # 4 q heads per core
QCOLS = HLOC * HD          # 256
NB = T // 512              # token nblocks per batch
KC = D // 128              # contraction chunks for projections
LEAD = 2                   # AV chunks lag scores by this many chunks

F32 = mybir.dt.float32
BF16 = mybir.dt.bfloat16

SWAP_MASK = [i ^ 1 for i in range(32)]  # adjacent pair swap per quadrant

EXP_FN = mybir.ActivationFunctionType.Exp


def build_nc():
    nc = bacc.Bacc(None, target_bir_lowering=False, debug=False)

    xT = nc.dram_tensor("xT", [D, B * T], BF16, kind="ExternalInput")
    wq_d = nc.dram_tensor("wq", [D, QCOLS], BF16, kind="ExternalInput")
    wkv_d = nc.dram_tensor("wkv", [D, 2 * HD], BF16, kind="ExternalInput")
    wo_d = nc.dram_tensor("wo", [QCOLS, D], BF16, kind="ExternalInput")
    rAq_d = nc.dram_tensor("ropeAq", [128, T], BF16, kind="ExternalInput")
    rBq_d = nc.dram_tensor("ropeBq", [128, T], BF16, kind="ExternalInput")
    rAkv_d = nc.dram_tensor("ropeAkv", [128, T], BF16, kind="ExternalInput")
    rBkv_d = nc.dram_tensor("ropeBkv", [128, T], BF16, kind="ExternalInput")
    out_d = nc.dram_tensor("out", [B * T, D], BF16, kind="ExternalOutput")

    with tile.TileContext(nc) as tc:
        with (
            tc.tile_pool(name="consts", bufs=1) as consts,
            tc.tile_pool(name="xp", bufs=36) as xp,
            tc.tile_pool(name="shufp", bufs=3) as shufp,
            tc.tile_pool(name="ropea", bufs=3) as ropea,
            tc.tile_pool(name="ropeb", bufs=3) as ropeb,
            tc.tile_pool(name="qropep", bufs=8) as qropep,
            tc.tile_pool(name="kvropep", bufs=2) as kvropep,
            tc.tile_pool(name="vextp", bufs=2) as vextp,
            tc.tile_pool(name="ptp", bufs=10) as ptp,
            tc.tile_pool(name="recipp", bufs=4) as recipp,
            tc.tile_pool(name="rbcp", bufs=3) as rbcp,
            tc.tile_pool(name="attp", bufs=4) as attp,
            tc.tile_pool(name="outp", bufs=6) as outp,
            tc.tile_pool(name="spp", bufs=3, space="PSUM") as spp,
            tc.tile_pool(name="avp", bufs=2, space="PSUM") as avp,
            tc.tile_pool(name="fillp", bufs=3, space="PSUM") as fillp,
        ):
            # ---- constants ----
            wq_sb = consts.tile([128, KC, QCOLS], BF16)
            nc.sync.dma_start(
                out=wq_sb[:], in_=wq_d.rearrange("(kc p) m -> p kc m", p=128)
            )
            wkv_sb = consts.tile([128, KC, 2 * HD], BF16)
            nc.sync.dma_start(
                out=wkv_sb[:], in_=wkv_d.rearrange("(kc p) m -> p kc m", p=128)
            )
            # wo/rope tiles are allocated here but their DMAs are issued
            # after the first x-tile batch (they aren't needed immediately
            # and would delay the first projection matmuls)
            wo_sb = consts.tile([128, 2, D], BF16)
            rAq = consts.tile([128, T], BF16)
            rBq = consts.tile([128, T], BF16)
            rAkv = consts.tile([128, T], BF16)
            rBkv = consts.tile([128, T], BF16)

            def emit_late_const_dmas():
                nc.sync.dma_start(out=rAq[:], in_=rAq_d[:])
                nc.sync.dma_start(out=rBq[:], in_=rBq_d[:])
                nc.sync.dma_start(out=rAkv[:], in_=rAkv_d[:])
                nc.sync.dma_start(out=rBkv[:], in_=rBkv_d[:])
                nc.sync.dma_start(
                    out=wo_sb[:], in_=wo_d.rearrange("(g p) n -> p g n", p=128)
                )

            ident = consts.tile([128, 128], BF16)
            make_identity(nc, ident[:])
            # mnegT[q, k] = -30000 if k > q else 0 — stationary operand of the
            # causal-mask matmul (adds -30000 above the diagonal of scores)
            mnegT = consts.tile([128, 128], BF16)
            nc.gpsimd.memset(mnegT[:], -30000.0)
            nc.gpsimd.affine_select(
                out=mnegT[:],
                in_=mnegT[:],
                compare_op=mybir.AluOpType.is_ge,
                fill=0.0,
                base=-1,
                pattern=[[1, 128]],
                channel_multiplier=-1,
            )
            # ident_ext: [I | 0] — moving operand of the causal-mask matmul
            ident_ext = consts.tile([128, 512], BF16)
            nc.gpsimd.memset(ident_ext[:], 0.0)
            nc.gpsimd.tensor_copy(ident_ext[:, 0:128], ident[:])


            # ---- per-batch persistent tiles ----
            qrope = {}   # (b, h) -> [64, T]
            kvrope = {}  # b -> [128, T]
            v_ext = {}   # b -> [128, KC, HD+1]
            attT = {}    # (b, g) -> [128, T]
            xts = {}     # (b, nb) -> list of 16 [128,512] tiles

            def alloc_batch(b):
                for h in range(HLOC):
                    qrope[(b, h)] = qropep.tile([64, T], BF16, tag="qrope", name=f"qr{b}{h}")
                kvrope[b] = kvropep.tile([128, T], BF16, tag="kvrope", name=f"kv{b}")
                for g in range(2):
                    attT[(b, g)] = attp.tile([128, T], BF16, tag="attT", name=f"at{b}{g}")

            alloc_batch(0)
            alloc_batch(1)

            # ---------- filler item emitters ----------
            def emit_dma_x(b, nb):
                tiles = []
                c0 = b * T + nb * 512
                for kc in range(KC):
                    xt = xp.tile([128, 512], BF16, tag="x", name="x")
                    nc.sync.dma_start(
                        out=xt[:], in_=xT[kc * 128 : (kc + 1) * 128, c0 : c0 + 512]
                    )
                    tiles.append(xt)
                xts[(b, nb)] = tiles

            def emit_proj_half(b, nb, mt, half, ps):
                # mt: 0,1 -> q cols [128*mt,128*mt+128); 2 -> kv
                tiles = xts[(b, nb)]
                for kc in range(half * 8, half * 8 + 8):
                    if mt < 2:
                        lhsT = wq_sb[:, kc, mt * 128 : (mt + 1) * 128]
                    else:
                        lhsT = wkv_sb[:, kc, :]
                    nc.tensor.matmul(
                        ps[:],
                        lhsT,
                        tiles[kc][:],
                        start=(kc == 0),
                        stop=(kc == KC - 1),
                    )

            def emit_rope_evict(b, nb, mt, ps):
                A = rAq if mt < 2 else rAkv
                Bp = rBq if mt < 2 else rBkv
                sl = slice(nb * 512, (nb + 1) * 512)
                tmp = shufp.tile([128, 512], F32, tag="shuf", name="shuf")
                nc.vector.stream_shuffle(tmp[:], ps[:], SWAP_MASK)
                t2 = ropea.tile([128, 512], BF16, tag="ra", name="ra")
                nc.vector.tensor_mul(t2[:], ps[:], A[:, sl])
                t3 = ropeb.tile([128, 512], BF16, tag="rb", name="rb")
                nc.gpsimd.tensor_mul(t3[:], tmp[:], Bp[:, sl])
                if mt < 2:
                    nc.gpsimd.tensor_add(
                        qrope[(b, 2 * mt)][:, sl], t2[0:64, :], t3[0:64, :]
                    )
                    nc.gpsimd.tensor_add(
                        qrope[(b, 2 * mt + 1)][:, sl], t2[64:128, :], t3[64:128, :]
                    )
                else:
                    nc.gpsimd.tensor_add(kvrope[b][:, sl], t2[:], t3[:])

            def emit_vext_nb(b, nb):
                # transpose V rows of this nb's 4 key chunks into one PSUM
                # bank, then evict with a single strided copy
                ve = v_ext[b]
                tp = fillp.tile([128, 4, HD], BF16, tag="fill", name="tpv")
                for jj in range(4):
                    j = 4 * nb + jj
                    nc.tensor.transpose(
                        tp[:, jj, :],
                        kvrope[b][HD:128, j * 128 : (j + 1) * 128],
                        ident[HD:128, HD:128],
                    )
                nc.scalar.activation(
                    ve[:, 4 * nb : 4 * nb + 4, 0:HD],
                    tp[:],
                    mybir.ActivationFunctionType.Copy,
                )

            _evict_flip = [0]

            def emit_outproj_tile(b, mt, nb2):
                op = fillp.tile([128, 512], F32, tag="fill", name="op")
                for g in range(2):
                    nc.tensor.matmul(
                        op[:],
                        attT[(b, g)][:, mt * 128 : (mt + 1) * 128],
                        wo_sb[:, g, nb2 * 512 : (nb2 + 1) * 512],
                        start=(g == 0),
                        stop=(g == 1),
                    )
                ot = outp.tile([128, 512], BF16, tag="ot", name="ot")
                if _evict_flip[0] % 4 == 3:
                    nc.scalar.activation(
                        ot[:], op[:], mybir.ActivationFunctionType.Copy
                    )
                else:
                    nc.vector.tensor_copy(ot[:], op[:])
                _evict_flip[0] += 1
                r0 = b * T + mt * 128
                nc.sync.dma_start(
                    out=out_d[r0 : r0 + 128, nb2 * 512 : (nb2 + 1) * 512],
                    in_=ot[:],
                )

            # ---------- filler queues ----------
            # each item: (pe_ns_estimate, fn)
            fill_proj = []   # must drain before attention of that batch
            fill_op = []     # output-projection tiles, can linger

            def build_proj_items(b):
                # items: (pe_ns, fn, stage) — attention(b, qb) may start once
                # all items with stage <= b*NB + qb have been emitted
                items = []
                chains = {}

                def mk_dma(nb, stage):
                    return (100.0, (lambda nb=nb: emit_dma_x(b, nb)), stage)

                def mk_half(nb, mt, half):
                    def f(nb=nb, mt=mt, half=half):
                        if (nb, mt) not in chains:
                            chains[(nb, mt)] = fillp.tile(
                                [128, 512], F32, tag="fill", name="pj"
                            )
                        emit_proj_half(b, nb, mt, half, chains[(nb, mt)])
                    return (8 * 512 * 0.42, f, b * NB + nb)

                def mk_evict(nb, mt):
                    def f(nb=nb, mt=mt):
                        emit_rope_evict(b, nb, mt, chains.pop((nb, mt)))
                    return (50.0, f, b * NB + nb)

                def mk_vext(nb):
                    return (300.0, (lambda nb=nb: emit_vext_nb(b, nb)), b * NB + nb)

                items.append(mk_dma(0, b * NB))
                if b == 0:
                    items.append((100.0, emit_late_const_dmas, 0))
                items.append(mk_dma(1, b * NB))
                for nb in range(NB):
                    if nb >= 1 and nb + 1 < NB:
                        items.append(mk_dma(nb + 1, b * NB + nb))
                    # kv chain first so its rope adds drain while q0/q1 run,
                    # letting the v_ext transposes start stall-free
                    for mt in (2, 0, 1):
                        items.append(mk_half(nb, mt, 0))
                        items.append(mk_half(nb, mt, 1))
                        items.append(mk_evict(nb, mt))
                    items.append(mk_vext(nb))
                return items

            deficit = [0.0]

            def pull():
                # emit filler items worth ~deficit ns of PE time
                budget = deficit[0]
                while (fill_proj or fill_op) and budget > 0:
                    if fill_proj:
                        pe_ns, fn, _ = fill_proj[0]
                        q = fill_proj
                    else:
                        pe_ns, fn = fill_op[0]
                        q = fill_op
                    if pe_ns > budget + 400:
                        break
                    q.pop(0)
                    fn()
                    budget -= pe_ns
                    deficit[0] -= pe_ns

            def drain_proj_stage(stage):
                while fill_proj and fill_proj[0][2] <= stage:
                    _, fn, _ = fill_proj.pop(0)
                    fn()
                    if fill_op:
                        pe_ns, fn2 = fill_op.pop(0)
                        fn2()
                        deficit[0] -= pe_ns

            # ---------- attention ----------
            def emit_attention(b):
                ve = v_ext[b]
                kvr = kvrope[b]
                pending_norm = []
                norms_done = [0]

                def flush_one_norm():
                    if not pending_norm:
                        return
                    pending_norm.pop(0)()
                    norms_done[0] += 1
                    if norms_done[0] % HLOC == 0:
                        qb_c = norms_done[0] // HLOC - 1
                        for mt in range(4 * qb_c, 4 * qb_c + 4):
                            for nb2 in range(NB):
                                fill_op.append(
                                    (2 * 512 * 0.42 + 80,
                                     lambda b=b, mt=mt, nb2=nb2:
                                         emit_outproj_tile(b, mt, nb2))
                                )

                for qb in range(NB):
                    drain_proj_stage(b * NB + qb)
                    q0 = qb * 512
                    for h in range(HLOC):
                        qTh = qrope[(b, h)]
                        nch = 4 * qb + 4
                        av = avp.tile([128, 512], F32, tag="av", name="av")
                        pend = []

                        def emit_av(j, mc, pt):
                            nc.tensor.matmul(
                                av[0 : HD + 1, mc:512],
                                ve[:, j, :],
                                pt[:, mc:512],
                                start=(j == 0),
                                stop=(j == nch - 1),
                            )

                        for j in range(nch):
                            jj = j - 4 * qb
                            mc = 128 * jj if jj > 0 else 0
                            sp = spp.tile([128, 512], F32, tag="sp", name="sp")
                            if jj >= 0:
                                # causal mask: sp[k, mc:512] = -30000*(k > q)
                                nc.tensor.matmul(
                                    sp[:, mc:512],
                                    mnegT[:],
                                    ident_ext[:, 0 : 512 - mc],
                                    start=True,
                                    stop=False,
                                )
                            nc.tensor.matmul(
                                sp[:, mc:512],
                                kvr[0:HD, j * 128 : (j + 1) * 128],
                                qTh[:, q0 + mc : q0 + 512],
                                start=(jj < 0),
                                stop=True,
                            )
                            pt = ptp.tile([128, 512], BF16, tag="pt", name="pt")
                            nc.scalar.activation(
                                pt[:, mc:512], sp[:, mc:512], EXP_FN
                            )
                            pend.append((j, mc, pt))
                            cols = 512 - mc
                            deficit[0] += (0.83 * cols + 280) - (0.84 * cols + 60)
                            if j == 1:
                                flush_one_norm()
                            if len(pend) > LEAD:
                                emit_av(*pend.pop(0))
                            pull()
                        while pend:
                            emit_av(*pend.pop(0))
                        # normalize via fast reciprocal of the sums row
                        # (staged to SBUF: the approx reciprocal's bit tricks
                        # need SBUF fp32), then a rank-1 PE matmul broadcasts
                        # the reciprocal into the av bank's free rows 64:128.
                        # The reciprocal runs now (vector); the PE-side
                        # broadcast + final mul are deferred into the next
                        # head's chunk stream so the PE never waits on them.
                        rcs = recipp.tile([1, 512], F32, tag="recips", name="rcs")
                        nc.vector.tensor_copy(rcs[:], av[HD : HD + 1, :])
                        rct = recipp.tile([1, 512], F32, tag="recip", name="rc")
                        nc.vector.reciprocal_approx_fast(rct[:], rcs[:])
                        rb = rbcp.tile([HD, 512], F32, tag="rbc", name="rb")
                        nc.gpsimd.partition_broadcast(rb[:], rct[:], channels=HD)

                        def norm_fn(av=av, rb=rb, b=b, h=h, q0=q0):
                            dest = attT[(b, h // 2)][
                                HD * (h % 2) : HD * (h % 2) + HD, q0 : q0 + 512
                            ]
                            nc.vector.tensor_mul(dest, av[0:HD, :], rb[:])

                        pending_norm.append(norm_fn)
                while pending_norm:
                    flush_one_norm()

            # ---------- the global schedule ----------
            # One merged stream: attention(b, qb) is emitted as soon as the
            # projections for its token blocks are in; everything else
            # (later proj chains, output projections) fills PE idle slots.
            v_ext[0] = vextp.tile([128, KC, HD + 1], BF16, tag="vext", name="ve0")
            nc.gpsimd.memset(v_ext[0][:], 1.0)
            v_ext[1] = vextp.tile([128, KC, HD + 1], BF16, tag="vext", name="ve1")
            nc.gpsimd.memset(v_ext[1][:], 1.0)

            # P1: projection of batch 0 (drained fully before attention —
            # a merged stage-gated schedule measured worse: the per-qb
            # forced drains starve the scalar engine in bursts)
            fill_proj.extend(build_proj_items(0))
            drain_proj_stage(NB - 1)

            # P2: attention b0 + filler (proj b1, early outproj b0)
            fill_proj.extend(build_proj_items(1))
            emit_attention(0)
            while fill_proj:
                _, fn, _ = fill_proj.pop(0)
                fn()
                if fill_op:
                    _, fn2 = fill_op.pop(0)
                    fn2()

            # P3: attention b1 + remaining outproj; P4: drain
            emit_attention(1)
            while fill_op:
                _, fn = fill_op.pop(0)
                fn()

    nc.compile()
    return nc


_NC = None


def _get_nc():
    global _NC
    if _NC is None:
        _NC = build_nc()
    return _NC


def make_in_maps(x, freqs_cos, freqs_sin, wq, wk, wv, wo):
    npdt = ml_dtypes.bfloat16
    x = np.asarray(x, np.float32)
    freqs_cos = np.asarray(freqs_cos, np.float32)
    freqs_sin = np.asarray(freqs_sin, np.float32)
    wq = np.asarray(wq, np.float32)
    wk = np.asarray(wk, np.float32)
    wv = np.asarray(wv, np.float32)
    wo = np.asarray(wo, np.float32)

    xT = np.ascontiguousarray(x.reshape(B * T, D).T.astype(npdt))

    cosT = freqs_cos.T  # [32, T]
    sinT = freqs_sin.T
    A64 = np.empty((64, T), np.float32)
    A64[0::2] = cosT
    A64[1::2] = cosT
    B64 = np.empty((64, T), np.float32)
    B64[0::2] = -sinT
    B64[1::2] = sinT
    one64 = np.ones((64, T), np.float32)
    zero64 = np.zeros((64, T), np.float32)
    rAq = np.ascontiguousarray(np.concatenate([A64, A64], 0).astype(npdt))
    rBq = np.ascontiguousarray(np.concatenate([B64, B64], 0).astype(npdt))
    rAkv = np.ascontiguousarray(np.concatenate([A64, one64], 0).astype(npdt))
    rBkv = np.ascontiguousarray(np.concatenate([B64, zero64], 0).astype(npdt))

    scale = np.float32(1.0 / np.sqrt(HD))
    in_maps = []
    for c in range(NCORE):
        wq_c = np.ascontiguousarray((wq[:, c * QCOLS : (c + 1) * QCOLS] * scale).astype(npdt))
        wkv_c = np.ascontiguousarray(
            np.concatenate(
                [wk[:, c * HD : (c + 1) * HD], wv[:, c * HD : (c + 1) * HD]], 1
            ).astype(npdt)
        )
        wo_c = np.ascontiguousarray(wo[c * QCOLS : (c + 1) * QCOLS, :].astype(npdt))
        in_maps.append(
            {
                "xT": xT,
                "wq": wq_c,
                "wkv": wkv_c,
                "wo": wo_c,
                "ropeAq": rAq,
                "ropeBq": rBq,
                "ropeAkv": rAkv,
                "ropeBkv": rBkv,
            }
        )
    return in_maps


def run(in_maps, trace=False, **kwargs):
    nc = _get_nc()
    return run_bass_kernel_spmd(
        nc, in_maps, core_ids=list(range(NCORE)), trace=trace, **kwargs
    )


def kernel(x, freqs_cos, freqs_sin, wq, wk, wv, wo):
    in_maps = make_in_maps(x, freqs_cos, freqs_sin, wq, wk, wv, wo)
    res = run(in_maps)
    total = np.zeros((B * T, D), np.float32)
    for r in res.results:
        total += np.asarray(r["out"], np.float32)
    return total.reshape(B, T, D)
